# revision 58
# baseline (speedup 1.0000x reference)
"""Batched ChebConv (K=3) Trainium2 kernel.

Strategy (dst-node sharding, 8 cores):
  - Nodes padded to 10240 = 80 windows x 128. Core c owns windows
    [10c, 10c+10) = nodes [1280c, 1280c+1280), all B=8 batches.
  - All batches ride in the free dim: gather rows are [512] = (b, d).
  - Propagation P(h)[col] += norm_e * h[row]:
      host sorts edges by destination window, dedups sources per window
      and builds a merged scatter matrix
        S'[uslot, dst_local] = sum_{e: src=uslot, dst} 16*|norm_e|  (fp8)
      streamed sequentially from HBM. The device gathers each unique
      source row once (SWDGE indexed gather, int16 idxs, fp8 payload)
      and accumulates psum[128 dst, 512] += S'_chunk.T @ G_chunk with
      fp8 DoubleRow matmuls (two 128-slot chunks per instruction).
      psum holds -16*P (norms are negative; scale folded downstream).
  - Launch 1: Tx1 slices for all cores -> host assembles full Tx1.
    Launch 2: gathers from 8*Tx1 (fp8 payload pre-scale), Tx2 =
    2*P(Tx1) - x, then out = x@W0 + Tx1@W1 + Tx2@W2 + bias. The x/Tx1
    projection operands come from HBM pre-transposed d-major (host),
    only Tx2 is transposed on the PE. Output written d-major; host
    untransposes.
"""

import os
import numpy as np

NC_CORES = 8
NPW = 128  # nodes per window


# ----------------------------------------------------------------------------
# host-side prep
# ----------------------------------------------------------------------------

def _prep_edges(edge_index, edge_attr, n_nodes, n_windows):
    """Sort edges by destination window, dedup sources per window, and
    build the merged scatter matrices.

    Returns (CH, uniq_pad[NW, CH*128] int16, sp[NW, 128, CH, 128] f32)
    where sp[w, p, c, :] holds S'[c*128+p, :] = 16*|norm| merged per
    (unique-source slot, local dst). uniq_pad is 0-padded: padding slots
    gather row 0 harmlessly and their zero S' rows nullify the data
    (negative "skip" idxs would break the DMA completion semaphore count).
    """
    row = edge_index[0].astype(np.int64)
    col = edge_index[1].astype(np.int64)
    ea = edge_attr.astype(np.float64)

    deg = np.zeros(n_nodes, np.float64)
    np.add.at(deg, row, ea)
    deg = deg.astype(np.float32)
    dis = np.where(deg > 0, 1.0 / np.sqrt(deg), 0.0).astype(np.float32)
    nabs = dis[row] * edge_attr.astype(np.float32) * dis[col]  # |norm|

    # sort by (window, src): window grouping is required for the scatter;
    # src-sorting makes the HBM gather near-sequential and is what
    # np.unique returns anyway.
    w_of_edge = col // NPW
    order = np.lexsort((row, w_of_edge))
    cnt = np.bincount(w_of_edge, minlength=n_windows)
    srt_row = row[order]
    srt_col = col[order]
    srt_nabs = nabs[order]
    pos = np.concatenate([[0], np.cumsum(cnt)])

    uniqs, invs = [], []
    max_u = 0
    for w in range(n_windows):
        u, inv = np.unique(srt_row[pos[w] : pos[w + 1]], return_inverse=True)
        uniqs.append(u)
        invs.append(inv)
        max_u = max(max_u, len(u))
    ch = -(-max_u // 128)  # odd ch OK: trailing chunk uses a single matmul
    slots = ch * 128

    uniq_pad = np.zeros((n_windows, slots), np.int16)
    sp = np.zeros((n_windows, slots, NPW), np.float32)
    for w in range(n_windows):
        e0, e1 = int(pos[w]), int(pos[w + 1])
        u = uniqs[w]
        uniq_pad[w, : len(u)] = u
        np.add.at(
            sp[w],
            (invs[w], srt_col[e0:e1] - w * NPW),
            16.0 * srt_nabs[e0:e1],
        )
    # [w, slot, dst] -> [w, p=slot%128, c=slot//128, dst]
    sp = sp.reshape(n_windows, ch, 128, NPW).transpose(0, 2, 1, 3)
    return ch, uniq_pad, np.ascontiguousarray(sp)


def _wrap16(a):
    """Element i -> [i%16, i//16], replicated to 128 partitions."""
    n = a.shape[-1]
    w = a.reshape(*a.shape[:-1], n // 16, 16)
    w = np.swapaxes(w, -1, -2)  # [..., 16, n//16]
    return np.concatenate([w] * 8, axis=-2)  # [..., 128, n//16]


# ----------------------------------------------------------------------------
# device program
# ----------------------------------------------------------------------------

def _build_prog(ch, wpc, npad, bd, epilogue):
    """One SPMD program: per-core propagation over `wpc` windows of `ch`
    unique-source chunks; if `epilogue`, also Tx2 and the W-projection
    output stage."""
    from concourse import bacc, tile, library_config
    import concourse.mybir as mybir

    f32 = mybir.dt.float32
    bf16 = mybir.dt.bfloat16
    fp8 = mybir.dt.float8e4
    i16 = mybir.dt.int16
    mul = mybir.AluOpType.mult
    sub = mybir.AluOpType.subtract
    add = mybir.AluOpType.add
    drow = mybir.MatmulPerfMode.DoubleRow

    # chunks per dma_gather call (<=8: 1024 idxs; HW fails above ~1k).
    # Must be EVEN: DoubleRow chunk pairs may not straddle gather tiles.
    GSEG = 8
    segs = [GSEG] * (ch // GSEG)
    if ch % GSEG:
        segs.append(ch % GSEG)
    nown = wpc * NPW  # nodes owned per core
    # psum holds -16*P(h); launch-2 payload additionally pre-scaled by 8
    psum_scale = 1.0 / 16.0 if not epilogue else 1.0 / (16.0 * 8.0)

    nq = int(os.environ.get("CHEB_QUEUES", "4"))
    nc = bacc.Bacc(
        "TRN2",
        target_bir_lowering=False,
        debug=False,
        num_devices=NC_CORES,
        num_swdge_queues=nq,
    )

    srcg = nc.dram_tensor("srcg", [npad, bd], fp8, kind="ExternalInput")
    idx_d = nc.dram_tensor("idx", [wpc, 128, ch * 8], i16, kind="ExternalInput")
    sp_d = nc.dram_tensor("sp", [wpc, 128, ch, 128], fp8, kind="ExternalInput")
    if epilogue:
        ident_d = nc.dram_tensor("ident", [128, 128], f32, kind="ExternalInput")
        xt_d = nc.dram_tensor("xt", [wpc, 64, 1024], bf16, kind="ExternalInput")
        t1t_d = nc.dram_tensor("t1t", [wpc, 64, 1024], bf16, kind="ExternalInput")
        w_d = nc.dram_tensor("w", [3, 64, 64], bf16, kind="ExternalInput")
        bias_d = nc.dram_tensor("bias", [64, 1], f32, kind="ExternalInput")
        outt_d = nc.dram_tensor("outt", [wpc, 64, 1024], bf16, kind="ExternalOutput")
    else:
        tx1_d = nc.dram_tensor("tx1", [nown, bd], bf16, kind="ExternalOutput")

    with tile.TileContext(nc) as tc:
        nc.gpsimd.load_library(library_config.mlp)
        with (
            tc.tile_pool(name="const", bufs=1) as constp,
            tc.tile_pool(name="gat", bufs=24) as gatp,
            tc.tile_pool(name="gatr", bufs=3) as gatrp,
            tc.tile_pool(name="meta", bufs=4) as metap,
            tc.tile_pool(name="spp", bufs=4) as spp,
            tc.tile_pool(name="outp", bufs=3) as outp,
            tc.tile_pool(name="ps", bufs=3 if epilogue else 4, space="PSUM") as psp,
            tc.tile_pool(name="tps", bufs=1 if epilogue else 2, space="PSUM") as tpsp,
            tc.tile_pool(name="ops", bufs=1, space="PSUM") as opsp,
        ):
            if epilogue:
                ident_t = constp.tile([128, 128], f32, tag="ident")
                nc.sync.dma_start(ident_t[:], ident_d[:])
                w_t = constp.tile([64, 3, 64], bf16, tag="w")
                nc.sync.dma_start(w_t[:], w_d.ap().rearrange("k d e -> d k e"))
                bias_t = constp.tile([64, 1], f32, tag="bias")
                nc.sync.dma_start(bias_t[:], bias_d[:])

            # dummy warm-up gather: pays the SWDGE ucode cold-start cost
            # (~8us) concurrently with the first window's metadata DMA
            widx_t = constp.tile([128, 8], i16, tag="widx")
            nc.gpsimd.memset(widx_t[:], 0)
            warm_t = constp.tile([128, 1, bd], fp8, tag="warm")
            nc.gpsimd.dma_gather(
                warm_t[:], srcg.ap(), widx_t[:], 128, 128, bd, queue_num=0
            )

            gq = [1]  # global gather-call counter for queue round-robin
            for w in range(wpc):
                idx_t = metap.tile([128, ch * 8], i16, tag="idx")
                nc.sync.dma_start(idx_t[:], idx_d[w])

                g_ts = []
                c0 = 0
                for seg in segs:
                    pool = gatp if seg == GSEG else gatrp
                    g_t = pool.tile(
                        [128, seg, bd], fp8, tag="g" if seg == GSEG else "gr"
                    )
                    nc.gpsimd.dma_gather(
                        g_t[:],
                        srcg.ap(),
                        idx_t[:, c0 * 8 : (c0 + seg) * 8],
                        seg * 128,
                        seg * 128,
                        bd,
                        queue_num=gq[0] % nq,
                    )
                    gq[0] += 1
                    g_ts.append(g_t)
                    c0 += seg
                sp_t = spp.tile([128, ch, 128], fp8, tag="sp")
                nc.sync.dma_start(sp_t[:], sp_d[w])
                ps = psp.tile([128, bd], f32, tag="acc")
                for c in range(0, ch - 1, 2):
                    h, cc = divmod(c, GSEG)
                    nc.tensor.matmul(
                        ps[:],
                        sp_t[:, c : c + 2, :],
                        g_ts[h][:, cc : cc + 2, :],
                        start=(c == 0),
                        stop=(ch % 2 == 0 and c == ch - 2),
                        perf_mode=drow,
                    )
                if ch % 2:  # odd trailing chunk: plain fp8 matmul
                    h, cc = divmod(ch - 1, GSEG)
                    nc.tensor.matmul(
                        ps[:],
                        sp_t[:, ch - 1, :],
                        g_ts[h][:, cc, :],
                        start=(ch == 1),
                        stop=True,
                    )

                if not epilogue:
                    o_t = outp.tile([128, bd], bf16, tag="o")
                    nc.vector.tensor_scalar(o_t[:], ps[:], -psum_scale, None, op0=mul)
                    nc.sync.dma_start(tx1_d[w * NPW : (w + 1) * NPW, :], o_t[:])
                else:
                    t2w = outp.tile([128, bd], f32, tag="t2w")
                    # 2*P(Tx1); the "- x" of Tx2 is folded into W0' = W0 - W2
                    nc.vector.tensor_scalar(
                        t2w[:], ps[:], -2.0 * psum_scale, None, op0=mul
                    )

                    # x/Tx1 arrive pre-transposed (d-major, bf16) from HBM;
                    # only Tx2 goes through PE transposes into PSUM plus one
                    # casting copy to SBUF, then per-quad N=512 bf16 matmuls
                    tsb = outp.tile([64, 3, 1024], bf16, tag="tsb")
                    nc.sync.dma_start(tsb[:, 0, :], xt_d[w])
                    nc.sync.dma_start(tsb[:, 1, :], t1t_d[w])
                    tps = tpsp.tile([64, 1024], f32, tag="tp")
                    for b in range(8):
                        nc.tensor.transpose(
                            tps[:, b * 128 : (b + 1) * 128],
                            t2w[:, b * 64 : (b + 1) * 64],
                            ident_t[:],
                        )
                    nc.vector.tensor_scalar(tsb[:, 2, :], tps[:], 1.0, None, op0=mul)
                    ops = opsp.tile([64, 1024], f32, tag="ot")
                    for q in range(2):
                        for k in range(3):
                            nc.tensor.matmul(
                                ops[:, q * 512 : (q + 1) * 512],
                                w_t[:, k, :],
                                tsb[:, k, q * 512 : (q + 1) * 512],
                                start=(k == 0),
                                stop=(k == 2),
                            )
                    osb = outp.tile([64, 1024], bf16, tag="osb")
                    nc.vector.tensor_scalar(osb[:], ops[:], bias_t[:, 0:1], None, op0=add)
                    nc.sync.dma_start(outt_d[w], osb[:])
    nc.compile()
    return nc


# ----------------------------------------------------------------------------
# entry point
# ----------------------------------------------------------------------------

LAST_EXEC_NS = []


_LAUNCH_NO = [0]


def _launch(nc, in_maps, trace):
    from concourse.bass_utils import run_bass_kernel_spmd

    tmpdir = None
    base = os.environ.get("CHEB_TMPDIR")
    if base:
        _LAUNCH_NO[0] += 1
        tmpdir = os.path.join(base, f"l{_LAUNCH_NO[0]}")
        os.makedirs(tmpdir, exist_ok=True)
    return run_bass_kernel_spmd(
        nc, in_maps, list(range(len(in_maps))), trace=trace, tmpdir=tmpdir
    )


def kernel(x, edge_index, edge_attr, W, bias):
    import ml_dtypes
    import concourse.mybir as mybir

    trace = bool(int(os.environ.get("CHEB_TRACE", "0")))
    fp8np = mybir.dt.np(mybir.dt.float8e4)
    bf16np = ml_dtypes.bfloat16

    B, N, D = x.shape
    bd = B * D
    nw = -(-N // NPW)  # windows for real nodes
    nw = -(-nw // NC_CORES) * NC_CORES  # pad to multiple of cores
    wpc = nw // NC_CORES
    npad = nw * NPW
    nown = wpc * NPW

    ch, uniq_pad, sp_all = _prep_edges(edge_index, edge_attr, N, nw)

    # gather source: node-major, all batches contiguous
    xg = np.zeros((npad, bd), np.float32)
    xg[:N] = np.ascontiguousarray(x.transpose(1, 0, 2)).reshape(N, bd)

    idx_all = _wrap16(uniq_pad)  # [nw, 128, ch*8]
    sp_all = sp_all.astype(fp8np)  # [nw, 128, ch, 128]
    ident = np.eye(128, dtype=np.float32)

    core_ids = list(range(NC_CORES))

    # ---- launch 1: Tx1 = P(x) ----
    prog1 = _build_prog(ch, wpc, npad, bd, epilogue=False)
    xg_g = xg.astype(fp8np)
    in_maps1 = []
    for c in core_ids:
        ws = slice(c * wpc, (c + 1) * wpc)
        in_maps1.append(
            {
                "srcg": xg_g,
                "idx": np.ascontiguousarray(idx_all[ws]),
                "sp": np.ascontiguousarray(sp_all[ws]),
            }
        )
    r1 = _launch(prog1, in_maps1, trace)
    tx1 = np.concatenate(
        [r1.results[c]["tx1"] for c in core_ids], axis=0
    ).astype(np.float32)

    # ---- launch 2: Tx2 + projection epilogue ----
    prog2 = _build_prog(ch, wpc, npad, bd, epilogue=True)
    tx1_g = (tx1 * 8.0).astype(fp8np)
    # d-major (transposed) projection operands: [W, d, b*128+nl]
    xt_all = (
        xg.reshape(nw, NPW, B, D).transpose(0, 3, 2, 1).reshape(nw, D, B * NPW)
    )
    t1t_all = (
        tx1.reshape(nw, NPW, B, D).transpose(0, 3, 2, 1).reshape(nw, D, B * NPW)
    )
    # fold "- x*W2" (from Tx2 = 2*P(Tx1) - x) into the W0 projection
    wfold = np.stack([W[0] - W[2], W[1], W[2]]).astype(np.float32)
    in_maps2 = []
    for c in core_ids:
        ws = slice(c * wpc, (c + 1) * wpc)
        in_maps2.append(
            {
                "srcg": tx1_g,
                "idx": np.ascontiguousarray(idx_all[ws]),
                "sp": np.ascontiguousarray(sp_all[ws]),
                "ident": ident,
                "xt": np.ascontiguousarray(xt_all[ws]).astype(bf16np),
                "t1t": np.ascontiguousarray(t1t_all[ws]).astype(bf16np),
                "w": wfold.astype(bf16np),
                "bias": bias.astype(np.float32).reshape(64, 1),
            }
        )
    r2 = _launch(prog2, in_maps2, trace)

    global LAST_EXEC_NS
    LAST_EXEC_NS = [r1.exec_time_ns, r2.exec_time_ns]

    # outt[w, e, b*128+nl] = out[b, core*1280 + w*128 + nl, e]
    out = np.empty((B, npad, 64), np.float32)
    for c in core_ids:
        ot = r2.results[c]["outt"].astype(np.float32).reshape(wpc, 64, 8, 128)
        # -> [b, w, nl, e]
        ot = ot.transpose(2, 0, 3, 1).reshape(B, nown, 64)
        out[:, c * nown : (c + 1) * nown, :] = ot
    return out[:, :N, :]
